# revision 1
# baseline (speedup 1.0000x reference)
"""Causal self-attention Trainium2 kernel.

B=4, T=2048, C=1024, H=16 heads, D=64. 8 NeuronCores, tensor-parallel over
heads: core c owns heads {2c, 2c+1}. Host pre-transposes x to xT [C, B*T],
column-shards W_attn / row-shards W_proj, sums the 8 partial outputs.

Device kernel (per core, SPMD), software-pipelined one batch deep so the
PE-heavy qkv matmuls fill the gaps of the ACT-paced attention stream:
  qkv:  qkvT[384, T] = W_core.T @ xT  (bf16 matmuls, K=C in 8 chunks);
        rows: [qA qB | kA kB | vA vB], 64 each. Bias added on eviction (DVE).
        v rows are PE-transposed to token-major V with 64 appended ones
        columns, so the y^T matmul emits the softmax denominators
        replicated on psum partitions 64-127, row-aligned with y.
  attn: S^T layout: S^T[keys,queries] = k @ q^T via matmul(lhsT=kT_chunk,
        rhs=qT_block, fp32r); the two heads' K=64 matmuls are issued
        adjacently so the PE can run them concurrently in different row
        groups. exp on ACT over both heads at once ([128, 2, 512] psum);
        causal mask via per-head column-restricted gpsimd affine_select;
        diagonal chunks restrict all work to the live query range.
        y^T[d, queries] accumulated via matmul(lhsT=[V|1s], rhs=P^T, bf16);
        normalization is recip + row-aligned mul on DVE.
  proj: partial out[tokens, C] = y^T.T @ W_proj_rows (fp32r), DMA'd out;
        deferred one block so the PE has ready work during normalize.
"""

import sys

sys.path.insert(0, "/opt/trn_rl_repo")

from contextlib import ExitStack

import numpy as np

import concourse.bass as bass
import concourse.mybir as mybir
import concourse.tile as tile
from concourse import bacc
from concourse.bass_utils import run_bass_kernel_spmd
from concourse.masks import make_identity

F32 = mybir.dt.float32
F32R = mybir.dt.float32r
BF16 = mybir.dt.bfloat16
AF = mybir.ActivationFunctionType

B, T, C, H, D = 4, 2048, 1024, 16, 64
NCORES = 8
HPC = H // NCORES  # heads per core = 2
TOK = B * T  # 8192
QKVC = HPC * D  # per-core channels per q/k/v = 128
TB = 256  # token block for the qkv phase
NBB = T // TB  # qkv token blocks per batch = 8
QB = 512  # query block for attention
NKC = T // 128  # key chunks per batch = 16
SCALE = 1.0 / 8.0  # 1/sqrt(D)


def build_program():
    nc = bacc.Bacc(
        "TRN2",
        target_bir_lowering=False,
        debug=False,
        num_devices=NCORES,
    )
    xt_d = nc.dram_tensor("xt", [C, TOK], BF16, kind="ExternalInput").ap()
    wqkv_d = nc.dram_tensor("wqkv", [C, 3 * QKVC], BF16, kind="ExternalInput").ap()
    bqkv_d = nc.dram_tensor("bqkv", [3 * QKVC], F32, kind="ExternalInput").ap()
    wproj_d = nc.dram_tensor("wproj", [QKVC, C], F32R, kind="ExternalInput").ap()
    outp_d = nc.dram_tensor("outp", [TOK, C], F32, kind="ExternalOutput").ap()

    with tile.TileContext(nc) as tc:
        with ExitStack() as ctx, nc.allow_low_precision(reason="fp32r matmul inputs"):
            _body(ctx, tc, xt_d, wqkv_d, bqkv_d, wproj_d, outp_d)
    nc.compile()
    return nc


class _Kern:
    def __init__(self, ctx, tc, xt_d, wqkv_d, bqkv_d, wproj_d, outp_d):
        nc = tc.nc
        self.nc = nc
        self.tc = tc
        self.outp_d = outp_d

        self.const = ctx.enter_context(tc.tile_pool(name="const", bufs=1))
        self.persist = ctx.enter_context(tc.tile_pool(name="persist", bufs=1))
        self.xt_pool = ctx.enter_context(tc.tile_pool(name="xt", bufs=3))
        self.vtmp_pool = ctx.enter_context(tc.tile_pool(name="vtmp", bufs=3))
        self.pt_pool = ctx.enter_context(tc.tile_pool(name="pt", bufs=8))
        self.yt_pool = ctx.enter_context(tc.tile_pool(name="yt", bufs=3))
        self.out_pool = ctx.enter_context(tc.tile_pool(name="osb", bufs=4))
        self.small_pool = ctx.enter_context(tc.tile_pool(name="small", bufs=4))

        self.ps_s = ctx.enter_context(tc.tile_pool(name="ps_s", bufs=2, space="PSUM"))
        self.ps_y = ctx.enter_context(tc.tile_pool(name="ps_y", bufs=2, space="PSUM"))
        self.ps_mm = ctx.enter_context(tc.tile_pool(name="ps_mm", bufs=2, space="PSUM"))

        # --- constants ---
        c = self.const
        # weight loads ride the ACT HWDGE ring (nc.scalar) so they don't
        # serialize with the xt streaming loads on the SP ring; the first
        # K-chunk is split out so the PE can start quickly
        self.wqkv_s = c.tile([128, 8, 3 * QKVC], BF16, tag="wqkv", name="wqkv_s")
        wqkv_r = wqkv_d.rearrange("(kc p) m -> p kc m", p=128)
        for kc in range(8):
            nc.scalar.dma_start(self.wqkv_s[:, kc : kc + 1, :], wqkv_r[:, kc : kc + 1, :])
        self.bqkv_s = c.tile([128, 3], F32, tag="bqkv", name="bqkv_s")
        nc.scalar.dma_start(self.bqkv_s[:], bqkv_d.rearrange("(m p) -> p m", p=128))
        self.wproj_s = c.tile([128, C], F32R, tag="wproj", name="wproj_s")
        nc.scalar.dma_start(self.wproj_s[:], wproj_d[:])
        self.ident = c.tile([128, 128], F32, tag="ident", name="ident")
        make_identity(nc, self.ident[:])

        # persistent activations
        self.qT = self.persist.tile([128, TOK], F32R, tag="qT", name="qT")
        self.kT = self.persist.tile([128, TOK], F32R, tag="kT", name="kT")
        # token-major V (cols 0:D) + 64 replicated ones columns (cols D:2D):
        # the y^T matmul then yields the softmax denominator replicated on
        # psum partitions D..2D, row-aligned with y for the normalize mul
        self.vones = self.persist.tile(
            [128, B, HPC, NKC, 2 * D], BF16, tag="vones", name="vones"
        )
        nc.gpsimd.memset(self.vones[:, :, :, :, D : 2 * D], 1.0)
        self.xt_r = xt_d.rearrange("(kc p) t -> p kc t", p=128)

    def qkv_block(self, b, nb):
        """QKV + V-transpose for token block nb (TB tokens) of batch b."""
        nc = self.nc
        n = b * NBB + nb
        xt_t = self.xt_pool.tile([128, 8, TB], BF16, tag="xt", name=f"xt{n}")
        if n == 0:
            # cold start: split the first load so the PE can start sooner
            for kc in range(8):
                nc.sync.dma_start(
                    xt_t[:, kc, :], self.xt_r[:, kc, n * TB : (n + 1) * TB]
                )
        else:
            nc.sync.dma_start(xt_t[:], self.xt_r[:, :, n * TB : (n + 1) * TB])
        for m in range(3):  # q, k, v row chunks
            ps = self.ps_mm.tile([128, TB], F32, tag="mm", name=f"qkvp{n}_{m}")
            for kc in range(8):
                nc.tensor.matmul(
                    ps[:],
                    self.wqkv_s[:, kc, m * 128 : (m + 1) * 128],
                    xt_t[:, kc, :],
                    start=(kc == 0),
                    stop=(kc == 7),
                )
            if m < 2:
                dst = (self.qT if m == 0 else self.kT)[:, n * TB : (n + 1) * TB]
                nc.vector.tensor_scalar_add(dst, ps[:], self.bqkv_s[:, m : m + 1])
            else:
                vt = self.vtmp_pool.tile([128, TB], F32, tag="vt", name=f"vt{n}")
                nc.vector.tensor_scalar_add(vt[:], ps[:], self.bqkv_s[:, 2:3])
                j0 = (TB // 128) * nb
                for jj in range(TB // 128):
                    pst = self.ps_mm.tile([128, 128], F32, tag="mm", name=f"tr{n}_{jj}")
                    nc.tensor.transpose(
                        pst[:], vt[:, jj * 128 : (jj + 1) * 128], self.ident[:]
                    )
                    nc.vector.tensor_copy(
                        self.vones[:, b, :, j0 + jj, 0:D],
                        pst[:].rearrange("p (h d) -> p h d", h=HPC),
                    )

    def attn_block(self, b, qb):
        """Attention + proj for query block qb (QB queries) of batch b."""
        nc = self.nc
        q0 = b * T + qb * QB
        nj = (qb + 1) * (QB // 128)  # key chunks attended by this block
        psy = [
            self.ps_y.tile([2 * D, QB], F32, tag="psy", name=f"psy{b}_{qb}_{h}")
            for h in range(HPC)
        ]
        for j in range(nj):  # key chunks of 128
            k0 = b * T + j * 128
            # diagonal trimming: for a diagonal chunk at offset d, queries
            # f < 128*d attend to no key in this chunk, so restrict all work
            # to the query range [f0, QB)
            d = j - (nj - 4)
            # cap the restriction at 256 live queries: below that, fp32r
            # matmuls drop to 4 cyc/row and the "saved" columns cost more
            # than computing them (the mask zeroes them regardless)
            f0 = min(128 * d, QB - 256) if d > 0 else 0
            # one 2-bank psum tile holds both heads' S^T for this chunk;
            # the two K=64 matmuls use partitions 0-63 / 64-127 -> different
            # PE row groups, issued adjacently so they can run concurrently
            ps2 = self.ps_s.tile([128, HPC, QB], F32, tag="s2", name=f"s{b}_{qb}_{j}")
            for h in range(HPC):
                nc.tensor.matmul(
                    ps2[:, h, f0:QB],
                    self.kT[h * D : (h + 1) * D, k0 : k0 + 128],
                    self.qT[h * D : (h + 1) * D, q0 + f0 : q0 + QB],
                    start=True,
                    stop=True,
                )
            pt = self.pt_pool.tile([128, HPC, QB], BF16, tag="pt", name=f"pt{b}_{qb}_{j}")
            nc.scalar.activation(pt[:, :, f0:QB], ps2[:, :, f0:QB], AF.Exp, scale=SCALE)
            if d >= 0:
                # mask only the 128-column window straddling the diagonal,
                # per head so the first yT matmul isn't gated on both
                cols = min(QB, 128 * (d + 1))
                for h in range(HPC):
                    nc.gpsimd.affine_select(
                        out=pt[:, h, f0:cols],
                        in_=pt[:, h, f0:cols],
                        base=QB * qb - 128 * j + f0,
                        channel_multiplier=-1,
                        pattern=[[1, cols - f0]],
                        compare_op=mybir.AluOpType.is_ge,
                        fill=0.0,
                    )
            for h in range(HPC):
                nc.tensor.matmul(
                    psy[h][:, f0:QB],
                    self.vones[:, b, h, j, :],
                    pt[:, h, f0:QB],
                    start=(j == 0),
                    stop=(j == nj - 1),
                )
        # normalize into yt (d-major, both heads stacked)
        yt = self.yt_pool.tile([128, QB], F32R, tag="yt", name=f"yt{b}_{qb}")
        for h in range(HPC):
            rec = self.small_pool.tile([D, QB], F32, tag="rec", name=f"rec{b}_{qb}_{h}")
            nc.vector.reciprocal(rec[:], psy[h][D : 2 * D, :])
            nc.vector.tensor_mul(yt[h * D : (h + 1) * D, :], psy[h][0:D, :], rec[:])
        return yt

    def proj_block(self, b, qb, yt):
        """Projection + output DMA for query block qb of batch b."""
        nc = self.nc
        q0 = b * T + qb * QB
        for tt in range(QB // 128):
            osb = self.out_pool.tile([128, C], F32, tag="osb", name=f"o{b}_{qb}_{tt}")
            for ncol in range(C // 512):
                po = self.ps_mm.tile([128, 512], F32, tag="mm", name=f"po{b}_{qb}_{tt}_{ncol}")
                nc.tensor.matmul(
                    po[:],
                    yt[:, tt * 128 : (tt + 1) * 128],
                    self.wproj_s[:, ncol * 512 : (ncol + 1) * 512],
                    start=True,
                    stop=True,
                )
                nc.vector.tensor_copy(osb[:, ncol * 512 : (ncol + 1) * 512], po[:])
            r0 = q0 + tt * 128
            nc.sync.dma_start(self.outp_d[r0 : r0 + 128, :], osb[:])


def _body(ctx, tc, xt_d, wqkv_d, bqkv_d, wproj_d, outp_d):
    k = _Kern(ctx, tc, xt_d, wqkv_d, bqkv_d, wproj_d, outp_d)
    # Software pipeline one batch deep: attention(b) interleaves with the
    # independent qkv(b+1) blocks so the PE always has ready matmuls while
    # ACT paces the softmax. proj is deferred one attention block so the PE
    # has ready work while the softmax-normalize chain completes.
    # qkv(0) is the prologue; batches 1..B-1 form a queue drained 2 blocks
    # per attention slot for the first half, then 1, so every attention
    # stretch (including the last batch's) has PE-dense qkv filler.
    pending = None
    for nb in range(NBB):
        k.qkv_block(0, nb)
    queue = [(b, nb) for b in range(1, B) for nb in range(NBB)]
    qi = 0
    nslots = B * (T // QB)
    for s in range(nslots):
        b, qb = s // (T // QB), s % (T // QB)
        want = 2 if s < nslots // 2 else 1
        # never emit attn before its qkv blocks: need batch b block 2qb+1
        need = 0 if b == 0 else (b - 1) * NBB + 2 * qb + 2
        while qi < len(queue) and (qi < need or want > 0):
            k.qkv_block(*queue[qi])
            qi += 1
            want -= 1
        yt = k.attn_block(b, qb)
        if pending is not None:
            k.proj_block(*pending)
        pending = (b, qb, yt)
    while qi < len(queue):
        k.qkv_block(*queue[qi])
        qi += 1
    k.proj_block(*pending)


_CACHED_NC = None


def _get_nc():
    global _CACHED_NC
    if _CACHED_NC is None:
        _CACHED_NC = build_program()
    return _CACHED_NC


def make_in_maps(x, W_attn, b_attn, W_proj):
    x = np.ascontiguousarray(np.asarray(x, dtype=np.float32))
    W_attn = np.asarray(W_attn, dtype=np.float32)
    b_attn = np.asarray(b_attn, dtype=np.float32)
    W_proj = np.asarray(W_proj, dtype=np.float32)
    import ml_dtypes

    xt = np.ascontiguousarray(x.reshape(TOK, C).T.astype(ml_dtypes.bfloat16))
    in_maps = []
    for c in range(NCORES):
        s = c * QKVC
        wq = W_attn[:, s : s + QKVC]
        wk = W_attn[:, C + s : C + s + QKVC]
        wv = W_attn[:, 2 * C + s : 2 * C + s + QKVC]
        wqkv = np.ascontiguousarray(
            np.concatenate([wq, wk, wv], axis=1).astype(ml_dtypes.bfloat16)
        )
        bq = b_attn[s : s + QKVC]
        bk = b_attn[C + s : C + s + QKVC]
        bv = b_attn[2 * C + s : 2 * C + s + QKVC]
        bqkv = np.ascontiguousarray(np.concatenate([bq, bk, bv]))
        wproj = np.ascontiguousarray(W_proj[s : s + QKVC, :])
        in_maps.append({"xt": xt, "wqkv": wqkv, "bqkv": bqkv, "wproj": wproj})
    return in_maps


def run(x, W_attn, b_attn, W_proj, b_proj, trace=False, **kwargs):
    nc = _get_nc()
    in_maps = make_in_maps(x, W_attn, b_attn, W_proj)
    res = run_bass_kernel_spmd(
        nc, in_maps, core_ids=list(range(NCORES)), trace=trace, **kwargs
    )
    acc = res.results[0]["outp"].astype(np.float32, copy=True)
    for c in range(1, NCORES):
        acc += res.results[c]["outp"]
    acc += np.asarray(b_proj, dtype=np.float32)[None, :]
    out = acc.reshape(B, T, C)
    return out, res


def kernel(x, W_attn, b_attn, W_proj, b_proj):
    out, _ = run(x, W_attn, b_attn, W_proj, b_proj, trace=False)
    return out



# revision 39
# speedup vs baseline: 1.1052x; 1.1052x over previous
"""Causal self-attention Trainium2 kernel.

B=4, T=2048, C=1024, H=16 heads, D=64. 8 NeuronCores, tensor-parallel over
heads: core c owns heads {2c, 2c+1}. Host pre-transposes x to xT [C, B*T],
column-shards W_attn / row-shards W_proj, sums the 8 bf16 partial outputs.

Device kernel (per core, SPMD). The attention stream is paced by the exp
chain on ACT (~0.9us per key chunk), so every other engine's work is
emission-scheduled around it:
  qkv:  q,k rows [128ch, tok] = W.T @ xT (bf16, K=C in 8 chunks), bias on
        eviction; v computed token-major directly (out[tok, vch] with
        lhsT=x chunk) into per-key-chunk V tiles with a 65th ones column —
        no PE transpose, v bias folded into the yt eviction instead.
        Blocks are split into ~1us steps, each step evicting the PREVIOUS
        step's psum (so the DVE never head-of-line blocks on a fresh
        matmul), and a proportional pacer feeds steps into the attention
        key loops as PE bubble filler.
  attn: S^T[keys, queries] per key chunk via matmul(lhsT=kT, rhs=qT, bf16,
        K=64), exact causal trim. exp on ACT over both heads at once;
        causal mask via per-head gpsimd affine_select on the diagonal
        window. P@V in y-form: per 128-query subchunk,
        matmul(out[128q, 65], lhsT=P^T tile, rhs=[V|1]) accumulated over
        key chunks in psum; the 65th column accumulates the softmax
        denominator row-aligned with y. P@V of chunk j-1 rides behind S of
        chunk j. Normalize rides the psum eviction (tensor_scalar mult by
        recip(denominator), per-partition scalar), spread across the key
        loop as each query subchunk's accumulation completes; y chunks are
        PE-transposed (bf16 identity, 1cyc/row) to d-major.
  proj: partial out[tokens, C] = yT.T @ W_proj (bf16), interleaved into the
        next block's key loop as mm/evict half-steps; evictions split
        DVE/Pool; bf16 partials DMA'd out and summed on host.
"""

import sys

sys.path.insert(0, "/opt/trn_rl_repo")

from contextlib import ExitStack

import numpy as np

import concourse.bass as bass
import concourse.mybir as mybir
import concourse.tile as tile
from concourse import bacc
from concourse.bass_utils import run_bass_kernel_spmd
from concourse.masks import make_identity

F32 = mybir.dt.float32
BF16 = mybir.dt.bfloat16
AF = mybir.ActivationFunctionType

B, T, C, H, D = 4, 2048, 1024, 16, 64
NCORES = 8
HPC = H // NCORES  # heads per core = 2
TOK = B * T  # 8192
QKVC = HPC * D  # per-core channels per q/k/v = 128
TB = 256  # token block for the qkv phase
NBB = T // TB  # qkv token blocks per batch = 8
QB = 512  # query block for attention
NQC = QB // 128  # 128-query subchunks per block = 4
NKC = T // 128  # key chunks per batch = 16
SCALE = 1.0 / 8.0  # 1/sqrt(D)


def build_program():
    nc = bacc.Bacc(
        "TRN2",
        target_bir_lowering=False,
        debug=False,
        num_devices=NCORES,
    )
    xt_d = nc.dram_tensor("xt", [C, TOK], BF16, kind="ExternalInput").ap()
    wqkv_d = nc.dram_tensor("wqkv", [C, 3 * QKVC], BF16, kind="ExternalInput").ap()
    bqkv_d = nc.dram_tensor("bqkv", [3 * QKVC], F32, kind="ExternalInput").ap()
    wproj_d = nc.dram_tensor("wproj", [QKVC, C], BF16, kind="ExternalInput").ap()
    outp_d = nc.dram_tensor("outp", [TOK, C], BF16, kind="ExternalOutput").ap()

    with tile.TileContext(nc) as tc:
        with ExitStack() as ctx, nc.allow_low_precision(reason="bf16 matmul inputs"):
            _body(ctx, tc, xt_d, wqkv_d, bqkv_d, wproj_d, outp_d)
    nc.compile()
    return nc


class _Kern:
    def __init__(self, ctx, tc, xt_d, wqkv_d, bqkv_d, wproj_d, outp_d):
        nc = tc.nc
        self.nc = nc
        self.tc = tc
        self.outp_d = outp_d

        self.const = ctx.enter_context(tc.tile_pool(name="const", bufs=1))
        self.persist = ctx.enter_context(tc.tile_pool(name="persist", bufs=1))
        self.xt_pool = ctx.enter_context(tc.tile_pool(name="xt", bufs=6))
        self.pt_pool = ctx.enter_context(tc.tile_pool(name="pt", bufs=6))
        self.ytmp_pool = ctx.enter_context(tc.tile_pool(name="ytmp", bufs=6))
        self.yt_pool = ctx.enter_context(tc.tile_pool(name="yt", bufs=2))
        self.out_pool = ctx.enter_context(tc.tile_pool(name="osb", bufs=6))
        self.small_pool = ctx.enter_context(tc.tile_pool(name="small", bufs=8))

        self.ps_s = ctx.enter_context(tc.tile_pool(name="ps_s", bufs=2, space="PSUM"))
        self.ps_y = ctx.enter_context(tc.tile_pool(name="ps_y", bufs=1, space="PSUM"))
        self.ps_mm = ctx.enter_context(tc.tile_pool(name="ps_mm", bufs=2, space="PSUM"))

        # --- constants ---
        c = self.const
        # weight loads ride the ACT HWDGE ring (nc.scalar) so they don't
        # serialize with the xt streaming loads on the SP ring; the first
        # K-chunk is split out so the PE can start quickly
        self.wqkv_s = c.tile([128, 8, 3 * QKVC], BF16, tag="wqkv", name="wqkv_s")
        wqkv_r = wqkv_d.rearrange("(kc p) m -> p kc m", p=128)
        nc.scalar.dma_start(self.wqkv_s[:, 0:1, :], wqkv_r[:, 0:1, :])
        nc.scalar.dma_start(self.wqkv_s[:, 1:8, :], wqkv_r[:, 1:8, :])
        self.bqkv_s = c.tile([128, 3], F32, tag="bqkv", name="bqkv_s")
        nc.scalar.dma_start(self.bqkv_s[:], bqkv_d.rearrange("(m p) -> p m", p=128))
        self.wproj_s = c.tile([128, C], BF16, tag="wproj", name="wproj_s")
        nc.scalar.dma_start(self.wproj_s[:], wproj_d[:])
        self.identB = c.tile([128, 128], BF16, tag="identB", name="identB")
        make_identity(nc, self.identB[:])
        # causal mask for the diagonal 128x128 window: within the window the
        # global (query - key) offset is always col - partition, so one
        # lower-triangle keep-mask (col >= partition) serves every diagonal
        # chunk; applied as a bf16 multiply on DVE (2x mode, ~127ns)
        self.tri = c.tile([128, 128], BF16, tag="tri", name="tri")
        nc.gpsimd.memset(self.tri[:], 1.0)
        nc.gpsimd.affine_select(
            out=self.tri[:],
            in_=self.tri[:],
            base=0,
            channel_multiplier=-1,
            pattern=[[1, 128]],
            compare_op=mybir.AluOpType.is_ge,
            fill=0.0,
        )

        # persistent activations
        self.qT = self.persist.tile([128, TOK], BF16, tag="qT", name="qT")
        self.kT = self.persist.tile([128, TOK], BF16, tag="kT", name="kT")
        # token-major V per key chunk (cols 0:D per head) + ones column
        # (col D): the y-form P@V matmul then accumulates the softmax
        # denominator on psum column D, row-aligned with y
        self.vones = self.persist.tile(
            [128, B, NKC, HPC, D + 1], BF16, tag="vones", name="vones"
        )
        nc.gpsimd.memset(self.vones[:, :, :, :, D : D + 1], 1.0)
        self.xt_r = xt_d.rearrange("(kc p) t -> p kc t", p=128)

    def qkv_block_steps(self, b, nb):
        """QKV for token block nb of batch b as ~1us emission steps. Each
        step evicts the previous step's psum (already compute-complete by
        then) so the DVE never blocks waiting on a fresh matmul chain."""
        nc = self.nc
        n = b * NBB + nb
        st = {}

        def load():
            xt_t = self.xt_pool.tile([128, 8, TB], BF16, tag="xt", name=f"xt{n}")
            if n == 0:
                # cold start: peel off K-chunk 0 so the PE can start sooner
                # without paying 8 serial HWDGE issues
                nc.sync.dma_start(
                    xt_t[:, 0, :], self.xt_r[:, 0, n * TB : (n + 1) * TB]
                )
                nc.sync.dma_start(
                    xt_t[:, 1:8, :], self.xt_r[:, 1:8, n * TB : (n + 1) * TB]
                )
            else:
                nc.sync.dma_start(xt_t[:], self.xt_r[:, :, n * TB : (n + 1) * TB])
            st["xt"] = xt_t

        def mm_qk(m):
            ps = self.ps_mm.tile([128, TB], F32, tag="mm", name=f"qkp{n}_{m}")
            for kc in range(8):
                nc.tensor.matmul(
                    ps[:],
                    self.wqkv_s[:, kc, m * 128 : (m + 1) * 128],
                    st["xt"][:, kc, :],
                    start=(kc == 0),
                    stop=(kc == 7),
                )
            st[m] = ps

        def ev_qk(m):
            dst = (self.qT if m == 0 else self.kT)[:, n * TB : (n + 1) * TB]
            nc.vector.tensor_scalar_add(dst, st.pop(m)[:], self.bqkv_s[:, m : m + 1])

        def mm_v(tt):
            # v token-major directly: out[tok, vch] = x_chunk.T @ Wv_chunk
            ps = self.ps_mm.tile([128, QKVC], F32, tag="mm", name=f"vp{n}_{tt}")
            for kc in range(8):
                nc.tensor.matmul(
                    ps[:],
                    st["xt"][:, kc, tt * 128 : (tt + 1) * 128],
                    self.wqkv_s[:, kc, 2 * QKVC : 3 * QKVC],
                    start=(kc == 0),
                    stop=(kc == 7),
                )
            st[("v", tt)] = ps

        def ev_v(tt):
            kc = 2 * nb + tt
            nc.vector.tensor_copy(
                self.vones[:, b, kc, :, 0:D],
                st.pop(("v", tt))[:].rearrange("p (h d) -> p h d", h=HPC),
            )

        return load, [
            lambda: mm_qk(0),
            lambda: (ev_qk(0), mm_qk(1)),
            lambda: (ev_qk(1), mm_v(0)),
            lambda: (ev_v(0), mm_v(1)),
            lambda: (ev_v(1),),
        ]

    def _pv(self, b, qb, j, d, pt, psy):
        """y-form P@V for key chunk j: per (head, live query subchunk),
        out[128q, 65] += P^T_tile.T @ [V|1]."""
        nc = self.nc
        for h in range(HPC):
            for qc in range(max(d, 0), NQC):
                # exactly ONE start=True per psum bank: psum zeroing is
                # 2KB-region granular, so a start for qc>0 would re-mark the
                # whole bank pending-zero and clobber the other accumulators.
                # qc>0 groups accumulate onto pending-zero bytes (read as 0).
                nc.tensor.matmul(
                    psy[:, h, qc * 65 : qc * 65 + 65],
                    pt[:, h, qc * 128 : (qc + 1) * 128],
                    self.vones[:, b, j, h, :],
                    start=(j == 0 and qc == 0),
                    stop=(j == 4 * qb + qc),
                    skip_group_check=True,
                )

    def attn_block(self, s, b, qb, proj_halves, pacer):
        """Attention for query block qb (QB queries) of batch b.

        proj_halves: deque of (slot, thunk) projection half-steps from
        previous blocks; pacer(): emits qkv filler steps. Both interleave
        into the key loop as ready PE work behind the ACT-paced exp chain.
        Halves may spill one slot further, but anything two slots old must
        drain before this slot's ytT reuses its ring buffer.
        """
        nc = self.nc
        while proj_halves and proj_halves[0][0] <= s - 2:
            proj_halves.pop(0)[1]()
        q0 = b * T + qb * QB
        nj = (qb + 1) * NQC  # key chunks attended by this block
        psy = self.ps_y.tile([128, HPC, 512], F32, tag="y", name=f"psy{b}_{qb}")
        ytT = self.yt_pool.tile([128, QB], BF16, tag="yt", name=f"yt{b}_{qb}")
        st = {}

        def y_evict(qc):
            # qc's accumulation is complete (stop chunk was nj-4+qc, two
            # iterations ago): normalize on evict, then transpose to d-major
            rec = self.small_pool.tile(
                [128, HPC, 1], F32, tag="rec", name=f"rec{b}_{qb}_{qc}"
            )
            den = psy[:, :, qc * 65 + 64 : qc * 65 + 65]
            nc.vector.reciprocal(rec[:, :, :], den)
            ytmp = self.ytmp_pool.tile(
                [128, 128], BF16, tag="ytmp", name=f"ym{b}_{qb}_{qc}"
            )
            for h in range(HPC):
                nc.vector.tensor_scalar_mul(
                    ytmp[:, h * D : (h + 1) * D],
                    psy[:, h, qc * 65 : qc * 65 + 64],
                    rec[:, h, :],
                )
            pst = self.ps_mm.tile([128, 128], BF16, tag="mm", name=f"ytr{b}_{qb}_{qc}")
            nc.tensor.transpose(pst[:], ytmp[:], self.identB[:])
            st[qc] = pst

        def yt_finish(qc):
            # d-major eviction; v bias rides here: partitions are the (h,d)
            # v channels, and y_norm + b_v is exact post-normalization
            nc.vector.tensor_scalar_add(
                ytT[:, qc * 128 : (qc + 1) * 128], st.pop(qc)[:], self.bqkv_s[:, 2:3]
            )

        prev = None
        for j in range(nj):  # key chunks of 128
            k0 = b * T + j * 128
            # filler first: a wait-prone instruction stalls everything behind
            # it in the in-order PE queue, so ready work must precede it
            pacer()
            # exact causal trim: for a diagonal chunk at offset d, queries
            # below 128*d attend to no key in this chunk
            d = j - (nj - 4)
            f0 = 128 * d if d > 0 else 0
            ps2 = self.ps_s.tile([128, HPC, QB], F32, tag="s2", name=f"s{b}_{qb}_{j}")
            for h in range(HPC):
                nc.tensor.matmul(
                    ps2[:, h, f0:QB],
                    self.kT[h * D : (h + 1) * D, k0 : k0 + 128],
                    self.qT[h * D : (h + 1) * D, q0 + f0 : q0 + QB],
                    start=True,
                    stop=True,
                )
            pt = self.pt_pool.tile([128, HPC, QB], BF16, tag="pt", name=f"pt{b}_{qb}_{j}")
            nc.scalar.activation(pt[:, :, f0:QB], ps2[:, :, f0:QB], AF.Exp, scale=SCALE)
            if d >= 0:
                # mask only the 128-column window straddling the diagonal,
                # per head so the first P@V matmul isn't gated on both.
                # Pool (SBUF-only there, which is legal): keeps the DVE free
                # for psum evictions, which only DVE can do.
                for h in range(HPC):
                    nc.gpsimd.tensor_mul(
                        pt[:, h, f0 : f0 + 128],
                        pt[:, h, f0 : f0 + 128],
                        self.tri[:],
                    )
            # one-chunk software pipeline: P@V of chunk j-1 goes on the PE
            # behind S of chunk j, so the PE never waits on the exp chain
            if prev is not None:
                self._pv(b, qb, prev[0], prev[1], prev[2], psy)
            # spread the y eviction pipeline: qc's evict two iterations after
            # its stop chunk, the d-major eviction one more later
            if nj - 2 <= j < nj - 2 + NQC and j - (nj - 2) in range(NQC):
                y_evict(j - (nj - 2))
            if nj - 1 <= j and j - (nj - 1) in st:
                yt_finish(j - (nj - 1))
            if proj_halves:
                proj_halves.pop(0)[1]()
            prev = (j, d, pt)
        self._pv(b, qb, prev[0], prev[1], prev[2], psy)
        # in-loop iterations evicted qc=0,1 and finished qc=0; drain the rest
        for qc in range(2, NQC):
            y_evict(qc)
            yt_finish(qc - 1)
        yt_finish(NQC - 1)
        return ytT

    def proj_halves(self, b, qb, ytT):
        """Projection + output DMA per 128-token chunk, as mm/evict
        half-steps so each eviction lands an iteration after its matmul.
        Evictions alternate DVE/Pool; output DMAs ride the Pool SWDGE ring
        (two token chunks per DMA) so they never block xt loads on SP."""
        q0 = b * T + qb * QB
        halves = []
        st = {}

        def mm(tt, ncol):
            po = self.ps_mm.tile([128, 512], F32, tag="mm", name=f"po{b}_{qb}_{tt}_{ncol}")
            self.nc.tensor.matmul(
                po[:],
                ytT[:, tt * 128 : (tt + 1) * 128],
                self.wproj_s[:, ncol * 512 : (ncol + 1) * 512],
                start=True,
                stop=True,
            )
            st[(tt, ncol)] = po

        def ev(tt, ncol):
            pair = tt // 2
            if (pair, "osb") not in st:
                st[(pair, "osb")] = self.out_pool.tile(
                    [128, 2, C], BF16, tag="osb", name=f"o{b}_{qb}_{pair}"
                )
            osb = st[(pair, "osb")]
            # DVE: psum reads are illegal on GPSIMD
            self.nc.vector.tensor_copy(
                osb[:, tt % 2, ncol * 512 : (ncol + 1) * 512], st.pop((tt, ncol))[:]
            )
            if ncol == 1 and tt % 2 == 1:
                r0 = q0 + (tt - 1) * 128
                dst = self.outp_d[r0 : r0 + 256, :].rearrange(
                    "(two p) c -> p two c", p=128
                )
                self.nc.sync.dma_start(dst, st.pop((pair, "osb"))[:])

        # chain so exactly one po is in flight and each eviction lands one
        # iteration after its matmul: [mm0], [ev0, mm1], [ev1, mm2], ...
        units = [(tt, ncol) for tt in range(NQC) for ncol in range(2)]
        halves.append(lambda: mm(*units[0]))
        for i in range(1, len(units)):
            halves.append(lambda i=i: (ev(*units[i - 1]), mm(*units[i])))
        halves.append(lambda: ev(*units[-1]))
        return halves


def _body(ctx, tc, xt_d, wqkv_d, bqkv_d, wproj_d, outp_d):
    k = _Kern(ctx, tc, xt_d, wqkv_d, bqkv_d, wproj_d, outp_d)
    # qkv emission is step-granular: each token block is 5 steps of ~1us PE
    # work. A proportional pacer feeds steps into the ACT-paced attention key
    # loops so the PE never idles behind the exp chain; a per-slot `need`
    # check densely drains whatever the attended keys/queries require.
    steps = []  # (block_index, thunk)
    loads = []  # per-block xt DMA, prefetched ~2 blocks ahead of compute
    for b in range(B):
        for nb in range(NBB):
            ld, block_steps = k.qkv_block_steps(b, nb)
            loads.append(ld)
            for t in block_steps:
                steps.append((b * NBB + nb, t))
    si = 0  # next compute step to emit
    li = 0  # next load to emit

    def prefetch(upto_block):
        nonlocal li
        while li < len(loads) and li <= upto_block:
            loads[li]()
            li += 1

    def blocks_done():
        return steps[si][0] if si < len(steps) else B * NBB

    nslots = B * (T // QB)
    # JIT pacing against the need curve: block n must be fully emitted
    # before the first slot that attends to its tokens. Ramp linearly
    # across each slot's iterations (plus a small lookahead for DMA
    # latency) so filler lands INSIDE the ACT-paced attention stretches
    # instead of as dense pre-slot bursts that leave the PE idle later.
    spb = 5  # steps per block
    need_list = [
        (s // (T // QB)) * NBB + 2 * (s % (T // QB)) + 2 for s in range(nslots)
    ] + [B * NBB]
    targets = []
    for s in range(nslots):
        nj = 4 * ((s % (T // QB)) + 1)
        st0, st1 = spb * need_list[s], spb * need_list[s + 1]
        for jj in range(nj):
            targets.append(st0 + (st1 - st0) * (jj + 1) / nj)
    LOOK = 3
    state = {"it": 0}

    def pacer():
        nonlocal si
        tgt = targets[min(state["it"] + LOOK, len(targets) - 1)]
        state["it"] += 1
        while si < len(steps) and si < tgt:
            prefetch(steps[si][0] + 3)
            steps[si][1]()
            si += 1

    pending = []
    for s in range(nslots):
        b, qb = s // (T // QB), s % (T // QB)
        # attention (b, qb) touches keys/values up to chunk 4qb+3 and queries
        # up to token (qb+1)*QB of batch b: token blocks 0..2qb+1 of batch b
        need = need_list[s]
        while si < len(steps) and blocks_done() < need:
            prefetch(steps[si][0] + 3)
            steps[si][1]()
            si += 1
        ytT = k.attn_block(s, b, qb, pending, pacer)
        pending.extend((s, t) for t in k.proj_halves(b, qb, ytT))
    for _, t in pending:
        t()
    while si < len(steps):
        prefetch(steps[si][0] + 3)
        steps[si][1]()
        si += 1


_CACHED_NC = None


def _get_nc():
    global _CACHED_NC
    if _CACHED_NC is None:
        _CACHED_NC = build_program()
    return _CACHED_NC


def make_in_maps(x, W_attn, b_attn, W_proj):
    x = np.ascontiguousarray(np.asarray(x, dtype=np.float32))
    W_attn = np.asarray(W_attn, dtype=np.float32)
    b_attn = np.asarray(b_attn, dtype=np.float32)
    W_proj = np.asarray(W_proj, dtype=np.float32)
    import ml_dtypes

    xt = np.ascontiguousarray(x.reshape(TOK, C).T.astype(ml_dtypes.bfloat16))
    in_maps = []
    for c in range(NCORES):
        s = c * QKVC
        wq = W_attn[:, s : s + QKVC]
        wk = W_attn[:, C + s : C + s + QKVC]
        wv = W_attn[:, 2 * C + s : 2 * C + s + QKVC]
        wqkv = np.ascontiguousarray(
            np.concatenate([wq, wk, wv], axis=1).astype(ml_dtypes.bfloat16)
        )
        bq = b_attn[s : s + QKVC]
        bk = b_attn[C + s : C + s + QKVC]
        bv = b_attn[2 * C + s : 2 * C + s + QKVC]
        bqkv = np.ascontiguousarray(np.concatenate([bq, bk, bv]))
        wproj = np.ascontiguousarray(W_proj[s : s + QKVC, :].astype(ml_dtypes.bfloat16))
        in_maps.append({"xt": xt, "wqkv": wqkv, "bqkv": bqkv, "wproj": wproj})
    return in_maps


def run(x, W_attn, b_attn, W_proj, b_proj, trace=False, **kwargs):
    nc = _get_nc()
    in_maps = make_in_maps(x, W_attn, b_attn, W_proj)
    res = run_bass_kernel_spmd(
        nc, in_maps, core_ids=list(range(NCORES)), trace=trace, **kwargs
    )
    acc = res.results[0]["outp"].astype(np.float32, copy=True)
    for c in range(1, NCORES):
        acc += res.results[c]["outp"].astype(np.float32)
    acc += np.asarray(b_proj, dtype=np.float32)[None, :]
    out = acc.reshape(B, T, C)
    return out, res


def kernel(x, W_attn, b_attn, W_proj, b_proj):
    out, _ = run(x, W_attn, b_attn, W_proj, b_proj, trace=False)
    return out
